# revision 4
# baseline (speedup 1.0000x reference)
"""Trainium2 Bass kernel for nn_Criterion_32830730011569 (v3).

8 cores = (image b) x (H-half h). Each core streams ONE concatenated
[NPIX, 264] f32 tensor (por|true|occ|occt_onehot) in 9 chunks (small first
chunk to cut lead-in). Per chunk:
  ACT: exp(por)->bf16, true->bf16 cast with accum_out (= dice den partial),
       exp(occ)
  DVE: bf16 mask-mult, bf16 tree-sum for the softmax denominator Z,
       reciprocal, a = true*(1/Z), occ·onehot product (bf16)
  PE : C[0:96,q] += a^T @ [expm | occ·onehot]; ones column in the stationary
       makes row 96 the pixel-sum row -> occupancy x-term for free.
Window BCE gathers 96x49 elements via 2 indirect DMAs (host-computed offsets).
ln/softplus on ACT (Ln table), all Ln uses batched at the end (2 table loads).
"""
import sys

sys.path.insert(0, "/opt/trn_rl_repo")
import numpy as np

B, H, W, Q, E, M, K, WIN = 4, 192, 192, 160, 96, 96, 4, 7
NO_E = 0.1
HALF = H // 2
NPIX = HALF * W        # 18432
P = 128
J = NPIX // P          # 144
JCS = [6] + [18] * 7 + [12]          # per-chunk pixel-columns, sums to 144
SC = Q + E + K + K     # 264 stream cols: por | true | occ | occt_onehot
W49 = WIN * WIN
RQ = Q + K             # 164 rhs cols: expm | occ*onehot
LE = E + 1             # 97 stationary cols: true*rz | ones

# sm32 pack column layout
C_OFFT = 0
C_OFFB = 49
C_VAL = 98
C_HS = 147
C_IEL = 307
C_W = 309
C_LAB = 311
C_PTS = 313
C_CEN = 315
C_CHOL = 317
C_IND = 321
NS = C_IND + Q         # 481

_CACHE = {}


def _build_nc():
    import concourse.bass as bass
    import concourse.bacc as bacc
    import concourse.tile as tile
    from concourse import mybir

    f32 = mybir.dt.float32
    i32 = mybir.dt.int32
    bf16 = mybir.dt.bfloat16
    AF = mybir.ActivationFunctionType
    OP = mybir.AluOpType
    AX = mybir.AxisListType

    nc = bacc.Bacc("TRN2", target_bir_lowering=False, debug=False, num_devices=8)

    stream = nc.dram_tensor("stream", [NPIX, SC], f32, kind="ExternalInput")
    bin_sl = nc.dram_tensor("bin_sl", [NPIX, Q], f32, kind="ExternalInput")
    sm32 = nc.dram_tensor("sm32", [P, NS], f32, kind="ExternalInput")
    partials = nc.dram_tensor("partials", [1, 12], f32, kind="ExternalOutput")

    def bc(ap, pos, count):
        new = list(ap.ap)
        new.insert(pos, [0, count])
        return bass.AP(tensor=ap.tensor, offset=ap.offset, ap=new)

    from contextlib import ExitStack

    with tile.TileContext(nc) as tc, ExitStack() as ctx:
        sing = ctx.enter_context(tc.tile_pool(name="sing", bufs=1))
        spool = ctx.enter_context(tc.tile_pool(name="spool", bufs=3))
        epool = ctx.enter_context(tc.tile_pool(name="epool", bufs=2))
        ps = ctx.enter_context(tc.tile_pool(name="ps", bufs=1, space="PSUM"))

        smt = sing.tile([P, NS], f32)
        nc.sync.dma_start(out=smt[:], in_=sm32.ap())

        # window gathers: per-element indirect DMA (offsets from host)
        tvw = sing.tile([M, W49], f32)
        bww = sing.tile([M, W49], f32)
        stream_flat = bass.AP(tensor=stream.ap().tensor, offset=0,
                              ap=[[1, NPIX * SC], [1, 1]])
        bin_flat = bass.AP(tensor=bin_sl.ap().tensor, offset=0,
                           ap=[[1, NPIX * Q], [1, 1]])
        offT = smt[0:M, C_OFFT:C_OFFT + W49].bitcast(i32)
        offB = smt[0:M, C_OFFB:C_OFFB + W49].bitcast(i32)
        nc.gpsimd.indirect_dma_start(
            out=tvw[:], out_offset=None, in_=stream_flat,
            in_offset=bass.IndirectOffsetOnAxis(ap=offT, axis=0))
        nc.gpsimd.indirect_dma_start(
            out=bww[:], out_offset=None, in_=bin_flat,
            in_offset=bass.IndirectOffsetOnAxis(ap=offB, axis=0))

        indb = sing.tile([P, Q], bf16)
        nc.scalar.activation(out=indb[:], in_=smt[:, C_IND:C_IND + Q], func=AF.Copy)

        stats = sing.tile([P, 12], f32)
        nc.vector.memset(stats[:], 0.0)
        ones = sing.tile([P, 1], f32)
        nc.vector.memset(ones[:], 1.0)
        s4buf = sing.tile([P, J], f32)
        den_acc = sing.tile([P, len(JCS)], f32)
        res = sing.tile([1, 12], f32)
        nc.vector.memset(res[:], 0.0)

        stream_v = stream.ap().rearrange("(p j) c -> p j c", p=P)
        C_ps = ps.tile([LE, RQ], f32)

        # mid-loop side work: only Exp-table activations + DVE ops
        def emit_side_exp():
            ebw = sing.tile([M, W49], f32)
            nc.scalar.activation(out=ebw[:], in_=bww[:], func=AF.Exp)
            ebc = sing.tile([M, 2], f32)
            nc.scalar.activation(out=ebc[:], in_=smt[0:M, C_IEL:C_IEL + 2], func=AF.Exp)
            prw = sing.tile([M, W49], f32)
            nc.vector.tensor_tensor(out=prw[:], in0=bww[:], in1=tvw[:], op=OP.mult)
            # NLL DVE part (no ln yet)
            d2 = sing.tile([M, 2], f32)
            nc.vector.tensor_tensor(out=d2[:], in0=smt[0:M, C_PTS:C_PTS + 2],
                                    in1=smt[0:M, C_CEN:C_CEN + 2], op=OP.subtract)
            rr = sing.tile([M, 2], f32)
            nc.vector.reciprocal(out=rr[:], in_=smt[0:M, C_CHOL:C_CHOL + 2])
            zz = sing.tile([M, 2], f32)
            nc.vector.tensor_tensor(out=zz[:, 0:1], in0=d2[:, 0:1], in1=rr[:, 0:1], op=OP.mult)
            t1 = sing.tile([M, 1], f32)
            nc.vector.tensor_tensor(out=t1[:], in0=smt[0:M, C_CHOL + 2:C_CHOL + 3],
                                    in1=zz[:, 0:1], op=OP.mult)
            nc.vector.tensor_tensor(out=t1[:], in0=d2[:, 1:2], in1=t1[:], op=OP.subtract)
            nc.vector.tensor_tensor(out=zz[:, 1:2], in0=t1[:], in1=rr[:, 1:2], op=OP.mult)
            sqs = sing.tile([M, 2], f32)
            nc.vector.scalar_tensor_tensor(
                out=sqs[:], in0=zz[:], scalar=1.0, in1=zz[:],
                op0=OP.mult, op1=OP.mult, accum_out=stats[0:M, 8:9])
            ldet = sing.tile([M, 1], f32)
            nc.vector.tensor_tensor(out=ldet[:], in0=smt[0:M, C_CHOL:C_CHOL + 1],
                                    in1=smt[0:M, C_CHOL + 1:C_CHOL + 2], op=OP.mult)
            return ebw, ebc, prw, ldet

        side = {}
        JMAX = max(JCS)
        # manual double-buffer for the matmul stationary so its ones column
        # (pixel-sum row of C) is written once, not per chunk
        tb_bufs = []
        for i_ in range(2):
            t_ = sing.tile([P, LE, JMAX], bf16, name=f"tbuf{i_}")
            nc.vector.memset(t_[:, E:LE, :], 1.0)
            tb_bufs.append(t_)
        j0 = 0
        for c, JC in enumerate(JCS):
            sl = slice(j0, j0 + JC)
            j0 += JC
            stf = spool.tile([P, JMAX, SC], f32, tag="stream")
            st = stf[:, 0:JC, :]
            nc.sync.dma_start(out=st, in_=stream_v[:, sl, :])
            exf = epool.tile([P, JMAX, RQ], bf16, tag="exp")
            ex = exf[:, 0:JC, :]
            nc.scalar.activation(out=ex[:, :, 0:Q], in_=st[:, :, 0:Q], func=AF.Exp)
            # masked expm (in-place), matmul rhs cols 0:160
            nc.vector.tensor_tensor(out=ex[:, :, 0:Q], in0=ex[:, :, 0:Q],
                                    in1=bc(indb[:], 1, JC), op=OP.mult)
            # occ*onehot -> rhs cols 160:164
            nc.vector.tensor_tensor(out=ex[:, :, Q:RQ], in0=st[:, :, Q + E:Q + E + K],
                                    in1=st[:, :, Q + E + K:SC], op=OP.mult)
            # bf16 tree-sum of masked expm -> Z
            scrf = epool.tile([P, JMAX, Q // 2], bf16, tag="scr")
            scr = scrf[:, 0:JC, :]
            w_ = Q // 2
            nc.vector.tensor_tensor(out=scr[:, :, 0:w_], in0=ex[:, :, 0:w_],
                                    in1=ex[:, :, w_:Q], op=OP.add)
            while w_ > 5:
                h_ = w_ // 2
                nc.vector.tensor_tensor(out=scr[:, :, 0:h_], in0=scr[:, :, 0:h_],
                                        in1=scr[:, :, h_:w_], op=OP.add)
                w_ = h_
            Zf = epool.tile([P, JMAX], f32, tag="Z")
            Z = Zf[:, 0:JC]
            nc.vector.tensor_reduce(out=Z, in_=scr[:, :, 0:w_], axis=AX.X, op=OP.add)
            rzf = epool.tile([P, JMAX], bf16, tag="rz")
            rz = rzf[:, 0:JC]
            with nc.allow_low_precision(reason="bf16 1/Z; dice tolerance is loose"):
                nc.vector.reciprocal(out=rz, in_=Z)
            # tb is stored e-major [P, e, j] so the rz broadcast sits on the
            # middle AP dim (innermost stride-1 keeps the DVE 2x bf16 path)
            tb = tb_bufs[c % 2][:, :, 0:JC]
            nc.scalar.activation(out=tb[:, 0:E, :].rearrange("p e j -> p j e"),
                                 in_=st[:, :, Q:Q + E], func=AF.Copy,
                                 accum_out=den_acc[:, c:c + 1])
            nc.vector.tensor_tensor(out=tb[:, 0:E, :], in0=tb[:, 0:E, :],
                                    in1=bc(rz, 1, E), op=OP.mult)
            for j in range(JC):
                nc.tensor.matmul(out=C_ps[:], lhsT=tb[:, :, j], rhs=ex[:, j, :],
                                 start=(c == 0 and j == 0),
                                 stop=(c == len(JCS) - 1 and j == JC - 1))
            # occ logsumexp partial: s4 = sum_k exp(occ)
            eof = epool.tile([P, JMAX, K], f32, tag="eo")
            eo = eof[:, 0:JC, :]
            last_act = nc.scalar.activation(out=eo, in_=st[:, :, Q + E:Q + E + K], func=AF.Exp)
            nc.vector.tensor_reduce(out=s4buf[:, sl], in_=eo, axis=AX.X, op=OP.add)
            if c == 2:
                side.update(zip(("ebw", "ebc", "prw", "ldet"), emit_side_exp()))

        # ---------- tail: all Ln-table work ----------
        # Pin every Ln after the last in-loop activation so the scheduler
        # cannot interleave them with Exps (each mix costs an ACT_TABLE_LOAD).
        from concourse.tile import add_dep_helper

        def pin(inst):
            add_dep_helper(inst.ins, last_act.ins, reason="ln after all exps")
            return inst

        # occ logsumexp: sum_j ln(s4) per partition via ACT accumulator
        lse = sing.tile([P, J], f32)
        pin(nc.scalar.activation(out=lse[:], in_=s4buf[:], func=AF.Ln,
                                 accum_out=stats[:, 4:5]))
        # window BCE: softplus = ln(exp+1)
        spw = sing.tile([M, W49], f32)
        pin(nc.scalar.activation(out=spw[:], in_=side["ebw"][:], func=AF.Ln, bias=1.0))
        dfw = sing.tile([M, W49], f32)
        nc.vector.tensor_tensor(out=dfw[:], in0=spw[:], in1=side["prw"][:], op=OP.subtract)
        scrw = sing.tile([M, W49], f32)
        nc.vector.scalar_tensor_tensor(
            out=scrw[:], in0=dfw[:], scalar=1.0, in1=smt[0:M, C_VAL:C_VAL + W49],
            op0=OP.mult, op1=OP.mult, accum_out=stats[0:M, 1:2])
        # class loss
        spc = sing.tile([M, 2], f32)
        pin(nc.scalar.activation(out=spc[:], in_=side["ebc"][:], func=AF.Ln, bias=1.0))
        tc1 = sing.tile([M, 2], f32)
        nc.vector.scalar_tensor_tensor(
            out=tc1[:], in0=spc[:], scalar=1.0, in1=smt[0:M, C_W:C_W + 2],
            op0=OP.mult, op1=OP.mult, accum_out=stats[0:M, 6:7])
        tc2 = sing.tile([M, 2], f32)
        nc.vector.scalar_tensor_tensor(
            out=tc2[:], in0=smt[0:M, C_IEL:C_IEL + 2], scalar=1.0,
            in1=smt[0:M, C_LAB:C_LAB + 2],
            op0=OP.mult, op1=OP.mult, accum_out=stats[0:M, 7:8])
        # NLL: 0.5*(z0^2+z1^2) + log(2pi) + ln(l00*l11)
        lnd = sing.tile([M, 1], f32)
        pin(nc.scalar.activation(out=lnd[:], in_=side["ldet"][:], func=AF.Ln))
        hq = sing.tile([M, 1], f32)
        nc.vector.tensor_scalar(out=hq[:], in0=stats[0:M, 8:9], scalar1=0.5,
                                scalar2=float(np.log(2.0 * np.pi)),
                                op0=OP.mult, op1=OP.add)
        nc.vector.tensor_tensor(out=stats[0:M, 0:1], in0=hq[:], in1=lnd[:], op=OP.add)
        # dice den partials
        nc.vector.tensor_reduce(out=stats[:, 3:4], in_=den_acc[:], axis=AX.X, op=OP.add)
        # dice num: sum over matched pairs of C
        scr2 = sing.tile([E, Q], f32)
        nc.vector.scalar_tensor_tensor(
            out=scr2[:], in0=C_ps[0:E, 0:Q], scalar=1.0, in1=smt[0:E, C_HS:C_HS + Q],
            op0=OP.mult, op1=OP.mult, accum_out=stats[0:E, 2:3])
        # occ x-term total: C row 96, cols 160:164 -> stats[96, 5]
        nc.vector.tensor_reduce(out=stats[E:LE, 5:6], in_=C_ps[E:LE, Q:RQ],
                                axis=AX.X, op=OP.add)

        fin_ps = ps.tile([1, 12], f32)
        nc.tensor.matmul(out=fin_ps[:], lhsT=ones[:], rhs=stats[:], start=True, stop=True)
        nc.vector.tensor_copy(out=res[:], in_=fin_ps[:])
        nc.sync.dma_start(out=partials.ap(), in_=res[:])

    nc.compile()
    return nc


def _get_nc():
    if "nc" not in _CACHE:
        _CACHE["nc"] = _build_nc()
    return _CACHE["nc"]


def make_in_maps(is_electron_logit, true_segmap, binary_mask_logits, portion_logits,
                 incidence_points, positions, chol, occupancy_logits, occupancy_true,
                 matched_q, matched_e):
    f = np.float32
    eye4 = np.eye(K, dtype=f)
    dr7 = np.arange(WIN) - WIN // 2
    in_maps = []
    for c in range(8):
        b, h = c // 2, c % 2
        sl = slice(h * HALF, (h + 1) * HALF)
        me = np.asarray(matched_e[b]).astype(np.int64)
        mq = np.asarray(matched_q[b]).astype(np.int64)

        por = np.asarray(portion_logits[b, sl], dtype=f).reshape(NPIX, Q)
        tru = np.asarray(true_segmap[b, sl], dtype=f).reshape(NPIX, E)
        occ = np.asarray(occupancy_logits[b, sl], dtype=f).reshape(NPIX, K)
        occt = np.asarray(occupancy_true[b, sl]).reshape(NPIX)
        stream = np.concatenate([por, tru, occ, eye4[occt]], axis=1)

        pts = np.asarray(incidence_points[b], dtype=f)[me]
        pix = np.floor(pts).astype(np.int64)
        rg = pix[:, 0:1] + dr7[None, :]
        cg = pix[:, 1:2] + dr7[None, :]
        valid = ((rg >= h * HALF) & (rg < (h + 1) * HALF)).astype(f)
        rl = rg - h * HALF
        flat = rl[:, :, None] * W + cg[:, None, :]
        flat = np.clip(flat, 0, NPIX - 1)
        offT = (flat * SC + Q + me[:, None, None]).astype(np.int32).reshape(M, W49)
        offB = (flat * Q + mq[:, None, None]).astype(np.int32).reshape(M, W49)
        valid49 = np.ascontiguousarray(
            np.broadcast_to(valid[:, :, None], (M, WIN, WIN))).reshape(M, W49)

        Hs = np.zeros((E, Q), dtype=f)
        Hs[me, mq] = 1.0
        ind = np.zeros(Q, dtype=f)
        ind[mq] = 1.0

        iel = np.asarray(is_electron_logit, dtype=f).reshape(B, Q)[b]
        lab = np.zeros(Q, dtype=f)
        lab[mq] = 1.0
        wgt = NO_E + (1.0 - NO_E) * lab

        def pack2(v):
            tmp = np.zeros(2 * M, dtype=f)
            tmp[:Q] = v
            return np.ascontiguousarray(tmp.reshape(2, M).T)

        iel2, lab2, w2 = pack2(iel), pack2(lab), pack2(wgt)

        chol_b = np.asarray(chol[b], dtype=f)[mq]
        cen = np.asarray(positions[b], dtype=f)[mq]

        sm = np.zeros((P, NS), dtype=f)
        sm[0:M, C_OFFT:C_OFFT + W49] = offT.view(f)
        sm[0:M, C_OFFB:C_OFFB + W49] = offB.view(f)
        sm[0:M, C_VAL:C_VAL + W49] = valid49
        sm[0:E, C_HS:C_HS + Q] = Hs
        sm[0:M, C_IEL:C_IEL + 2] = iel2
        sm[0:M, C_W:C_W + 2] = w2
        sm[0:M, C_LAB:C_LAB + 2] = lab2
        sm[0:M, C_PTS:C_PTS + 2] = pts
        sm[0:M, C_CEN:C_CEN + 2] = cen
        sm[0:M, C_CHOL + 0] = chol_b[:, 0, 0]
        sm[0:M, C_CHOL + 1] = chol_b[:, 1, 1]
        sm[0:M, C_CHOL + 2] = chol_b[:, 1, 0]
        sm[:, C_IND:C_IND + Q] = ind[None, :]

        in_maps.append(dict(
            stream=np.ascontiguousarray(stream),
            bin_sl=np.ascontiguousarray(binary_mask_logits[b, sl]).reshape(NPIX, Q),
            sm32=sm,
        ))
    return in_maps


def combine(partials_list):
    s = np.stack([np.asarray(p, dtype=np.float64).reshape(12) for p in partials_list])
    # slots: 0=nll 1=bce 2=num2 3=den_true 4=sum_lse 5=occ_xt 6=cls_sp 7=cls_xz
    class_loss = (s[0::2, 6].sum() - s[0::2, 7].sum()) / (B * Q)
    nll_loss = s[0::2, 0].sum() / (B * M)
    bce_loss = s[:, 1].sum() / (B * M * W49)
    occ_loss = (s[:, 4].sum() - s[:, 5].sum()) / (B * H * W)
    dice = 0.0
    for b in range(B):
        num = 2.0 * (s[2 * b, 2] + s[2 * b + 1, 2])
        den = s[2 * b, 3] + s[2 * b + 1, 3] + H * W
        dice += 1.0 - (num + 1.0) / (den + 1.0)
    dice_loss = dice / B
    return np.float32(class_loss + bce_loss + dice_loss + nll_loss + occ_loss)


def kernel(**inputs):
    from concourse.bass_utils import run_bass_kernel_spmd
    nc = _get_nc()
    in_maps = make_in_maps(**{k: np.asarray(v) for k, v in inputs.items()})
    r = run_bass_kernel_spmd(nc, in_maps, list(range(8)))
    return combine([r.results[c]["partials"] for c in range(8)])


# revision 9
# speedup vs baseline: 1.3448x; 1.3448x over previous
"""Trainium2 Bass kernel for nn_Criterion_32830730011569 (v3).

8 cores = (image b) x (H-half h). Each core streams ONE concatenated
[NPIX, 264] f32 tensor (por|true|occ|occt_onehot) in 9 chunks (small first
chunk to cut lead-in). Per chunk:
  ACT: exp(por)->bf16, true->bf16 cast with accum_out (= dice den partial),
       exp(occ)
  DVE: bf16 mask-mult, bf16 tree-sum for the softmax denominator Z,
       reciprocal, a = true*(1/Z), occ·onehot product (bf16)
  PE : C[0:96,q] += a^T @ [expm | occ·onehot]; ones column in the stationary
       makes row 96 the pixel-sum row -> occupancy x-term for free.
Window BCE gathers 96x49 elements via 2 indirect DMAs (host-computed offsets).
ln/softplus on ACT (Ln table), all Ln uses batched at the end (2 table loads).
"""
import sys

sys.path.insert(0, "/opt/trn_rl_repo")
import numpy as np

B, H, W, Q, E, M, K, WIN = 4, 192, 192, 160, 96, 96, 4, 7
NO_E = 0.1
HALF = H // 2
NPIX = HALF * W        # 18432
P = 128
J = NPIX // P          # 144
JCS = [6] + [18] * 7 + [12]          # per-chunk pixel-columns, sums to 144
SC = Q + E + K + K     # 264 stream cols: por | true | occ | occt_onehot
W49 = WIN * WIN
RQ = Q + K             # 164 rhs cols: expm | occ*onehot
LE = E + 1             # 97 stationary cols: true*rz | ones

# sm32 pack column layout
C_OFFT = 0
C_OFFB = 49
C_VAL = 98
C_HS = 147
C_IEL = 307
C_W = 309
C_LAB = 311
C_PTS = 313
C_CEN = 315
C_CHOL = 317
C_IND = 321
NS = C_IND + Q         # 481

_CACHE = {}


def _build_nc():
    import concourse.bass as bass
    import concourse.bacc as bacc
    import concourse.tile as tile
    from concourse import mybir

    f32 = mybir.dt.float32
    i32 = mybir.dt.int32
    bf16 = mybir.dt.bfloat16
    AF = mybir.ActivationFunctionType
    OP = mybir.AluOpType
    AX = mybir.AxisListType

    nc = bacc.Bacc("TRN2", target_bir_lowering=False, debug=False, num_devices=8)

    stream = nc.dram_tensor("stream", [NPIX, SC], f32, kind="ExternalInput")
    bin_sl = nc.dram_tensor("bin_sl", [NPIX, Q], f32, kind="ExternalInput")
    sm32 = nc.dram_tensor("sm32", [P, NS], f32, kind="ExternalInput")
    partials = nc.dram_tensor("partials", [1, 12], f32, kind="ExternalOutput")

    def bc(ap, pos, count):
        new = list(ap.ap)
        new.insert(pos, [0, count])
        return bass.AP(tensor=ap.tensor, offset=ap.offset, ap=new)

    from contextlib import ExitStack

    with tile.TileContext(nc) as tc, ExitStack() as ctx:
        sing = ctx.enter_context(tc.tile_pool(name="sing", bufs=1))
        spool = ctx.enter_context(tc.tile_pool(name="spool", bufs=3))
        epool = ctx.enter_context(tc.tile_pool(name="epool", bufs=2))
        ps = ctx.enter_context(tc.tile_pool(name="ps", bufs=1, space="PSUM"))

        smt = sing.tile([P, NS], f32)
        nc.scalar.dma_start(out=smt[:], in_=sm32.ap())

        # window gathers: per-element indirect DMA (offsets from host)
        tvw = sing.tile([M, W49], f32)
        bww = sing.tile([M, W49], f32)
        stream_flat = bass.AP(tensor=stream.ap().tensor, offset=0,
                              ap=[[1, NPIX * SC], [1, 1]])
        bin_flat = bass.AP(tensor=bin_sl.ap().tensor, offset=0,
                           ap=[[1, NPIX * Q], [1, 1]])
        offT = smt[0:M, C_OFFT:C_OFFT + W49].bitcast(i32)
        offB = smt[0:M, C_OFFB:C_OFFB + W49].bitcast(i32)
        nc.gpsimd.indirect_dma_start(
            out=tvw[:], out_offset=None, in_=stream_flat,
            in_offset=bass.IndirectOffsetOnAxis(ap=offT, axis=0))
        nc.gpsimd.indirect_dma_start(
            out=bww[:], out_offset=None, in_=bin_flat,
            in_offset=bass.IndirectOffsetOnAxis(ap=offB, axis=0))

        stats = sing.tile([P, 12], f32)
        nc.vector.memset(stats[:], 0.0)
        ones = sing.tile([P, 1], f32)
        nc.vector.memset(ones[:], 1.0)
        s4buf = sing.tile([P, J], f32)
        res = sing.tile([1, 12], f32)
        nc.vector.memset(res[:], 0.0)

        stream_v = stream.ap().rearrange("(p j) c -> p j c", p=P)
        C_ps = ps.tile([LE, RQ], f32)

        # mid-loop side work: only Exp-table activations + DVE ops
        def emit_side_exp():
            ebw = sing.tile([M, W49], f32)
            nc.scalar.activation(out=ebw[:], in_=bww[:], func=AF.Exp)
            ebc = sing.tile([M, 2], f32)
            nc.scalar.activation(out=ebc[:], in_=smt[0:M, C_IEL:C_IEL + 2], func=AF.Exp)
            prw = sing.tile([M, W49], f32)
            nc.vector.tensor_tensor(out=prw[:], in0=bww[:], in1=tvw[:], op=OP.mult)
            # NLL DVE part (no ln yet)
            d2 = sing.tile([M, 2], f32)
            nc.vector.tensor_tensor(out=d2[:], in0=smt[0:M, C_PTS:C_PTS + 2],
                                    in1=smt[0:M, C_CEN:C_CEN + 2], op=OP.subtract)
            rr = sing.tile([M, 2], f32)
            nc.vector.reciprocal(out=rr[:], in_=smt[0:M, C_CHOL:C_CHOL + 2])
            zz = sing.tile([M, 2], f32)
            nc.vector.tensor_tensor(out=zz[:, 0:1], in0=d2[:, 0:1], in1=rr[:, 0:1], op=OP.mult)
            t1 = sing.tile([M, 1], f32)
            nc.vector.tensor_tensor(out=t1[:], in0=smt[0:M, C_CHOL + 2:C_CHOL + 3],
                                    in1=zz[:, 0:1], op=OP.mult)
            nc.vector.tensor_tensor(out=t1[:], in0=d2[:, 1:2], in1=t1[:], op=OP.subtract)
            nc.vector.tensor_tensor(out=zz[:, 1:2], in0=t1[:], in1=rr[:, 1:2], op=OP.mult)
            sqs = sing.tile([M, 2], f32)
            nc.vector.scalar_tensor_tensor(
                out=sqs[:], in0=zz[:], scalar=1.0, in1=zz[:],
                op0=OP.mult, op1=OP.mult, accum_out=stats[0:M, 8:9])
            ldet = sing.tile([M, 1], f32)
            nc.vector.tensor_tensor(out=ldet[:], in0=smt[0:M, C_CHOL:C_CHOL + 1],
                                    in1=smt[0:M, C_CHOL + 1:C_CHOL + 2], op=OP.mult)
            return ebw, ebc, prw, ldet

        side = {}
        JMAX = max(JCS)
        # manual double-buffer for the matmul stationary so its ones column
        # (pixel-sum row of C) is written once, not per chunk
        tb_bufs = []
        for i_ in range(2):
            t_ = sing.tile([P, JMAX, LE], bf16, name=f"tbuf{i_}")
            nc.vector.memset(t_[:, :, E:LE], 1.0)
            tb_bufs.append(t_)
        j0 = 0
        for c, JC in enumerate(JCS):
            sl = slice(j0, j0 + JC)
            j0 += JC
            stf = spool.tile([P, JMAX, SC], f32, tag="stream")
            st = stf[:, 0:JC, :]
            nc.sync.dma_start(out=st, in_=stream_v[:, sl, :])
            exf = epool.tile([P, JMAX, RQ], bf16, tag="exp")
            ex = exf[:, 0:JC, :]
            scrf = epool.tile([P, JMAX, Q // 2], bf16, tag="scr")
            Zf = epool.tile([P, JMAX], f32, tag="Z")
            rzf = epool.tile([P, JMAX], f32, tag="rz")
            tbc = tb_bufs[c % 2]
            eof = epool.tile([P, JMAX, K], f32, tag="eo")
            # the last chunk is split into two j-halves so its ACT/DVE/PE
            # stages overlap during the pipeline drain
            halves = ([(0, JC)] if c < len(JCS) - 1 else
                      [(0, JC // 2), (JC // 2, JC)])
            for hi, (ja, jb) in enumerate(halves):
                js = slice(ja, jb)
                nc.scalar.activation(out=ex[:, js, 0:Q], in_=st[:, js, 0:Q], func=AF.Exp)
                # occ*onehot -> rhs cols 160:164
                nc.vector.tensor_tensor(out=ex[:, js, Q:RQ],
                                        in0=st[:, js, Q + E:Q + E + K],
                                        in1=st[:, js, Q + E + K:SC], op=OP.mult)
                # bf16 tree-sum of masked expm -> Z
                scr = scrf[:, js, :]
                exh = ex[:, js, :]
                w_ = Q // 2
                nc.vector.tensor_tensor(out=scr[:, :, 0:w_], in0=exh[:, :, 0:w_],
                                        in1=exh[:, :, w_:Q], op=OP.add)
                while w_ > 10:
                    h_ = w_ // 2
                    nc.vector.tensor_tensor(out=scr[:, :, 0:h_], in0=scr[:, :, 0:h_],
                                            in1=scr[:, :, h_:w_], op=OP.add)
                    w_ = h_
                Z = Zf[:, js]
                nc.vector.tensor_reduce(out=Z, in_=scr[:, :, 0:w_], axis=AX.X, op=OP.add)
                rz = rzf[:, js]
                nc.vector.reciprocal(out=rz, in_=Z)
                # single mixed-dtype op: true(f32) * rz -> bf16 stationary
                nc.vector.tensor_tensor(out=tbc[:, js, 0:E], in0=st[:, js, Q:Q + E],
                                        in1=bc(rz, 2, E), op=OP.mult)
                for j in range(ja, jb):
                    nc.tensor.matmul(out=C_ps[:], lhsT=tbc[:, j, :], rhs=ex[:, j, :],
                                     start=(c == 0 and j == 0),
                                     stop=(c == len(JCS) - 1 and j == JC - 1))
                # occ logsumexp partial: s4 = sum_k exp(occ)
                eo = eof[:, js, :]
                last_act = nc.scalar.activation(out=eo, in_=st[:, js, Q + E:Q + E + K],
                                                func=AF.Exp)
                nc.vector.tensor_reduce(out=s4buf[:, sl][:, js], in_=eo, axis=AX.X, op=OP.add)
            if c == 2:
                side.update(zip(("ebw", "ebc", "prw", "ldet"), emit_side_exp()))

        # ---------- tail: all Ln-table work ----------
        # Pin every Ln after the last in-loop activation so the scheduler
        # cannot interleave them with Exps (each mix costs an ACT_TABLE_LOAD).
        from concourse.tile import add_dep_helper

        def pin(inst):
            add_dep_helper(inst.ins, last_act.ins, reason="ln after all exps")
            return inst

        # occ logsumexp: sum_j ln(s4) per partition via ACT accumulator
        lse = sing.tile([P, J], f32)
        pin(nc.scalar.activation(out=lse[:], in_=s4buf[:], func=AF.Ln,
                                 accum_out=stats[:, 4:5]))
        # window BCE: softplus = ln(exp+1)
        spw = sing.tile([M, W49], f32)
        pin(nc.scalar.activation(out=spw[:], in_=side["ebw"][:], func=AF.Ln, bias=1.0))
        dfw = sing.tile([M, W49], f32)
        nc.vector.tensor_tensor(out=dfw[:], in0=spw[:], in1=side["prw"][:], op=OP.subtract)
        scrw = sing.tile([M, W49], f32)
        nc.vector.scalar_tensor_tensor(
            out=scrw[:], in0=dfw[:], scalar=1.0, in1=smt[0:M, C_VAL:C_VAL + W49],
            op0=OP.mult, op1=OP.mult, accum_out=stats[0:M, 1:2])
        # class loss
        spc = sing.tile([M, 2], f32)
        pin(nc.scalar.activation(out=spc[:], in_=side["ebc"][:], func=AF.Ln, bias=1.0))
        tc1 = sing.tile([M, 2], f32)
        nc.vector.scalar_tensor_tensor(
            out=tc1[:], in0=spc[:], scalar=1.0, in1=smt[0:M, C_W:C_W + 2],
            op0=OP.mult, op1=OP.mult, accum_out=stats[0:M, 6:7])
        tc2 = sing.tile([M, 2], f32)
        nc.vector.scalar_tensor_tensor(
            out=tc2[:], in0=smt[0:M, C_IEL:C_IEL + 2], scalar=1.0,
            in1=smt[0:M, C_LAB:C_LAB + 2],
            op0=OP.mult, op1=OP.mult, accum_out=stats[0:M, 7:8])
        # NLL: 0.5*(z0^2+z1^2) + log(2pi) + ln(l00*l11)
        lnd = sing.tile([M, 1], f32)
        pin(nc.scalar.activation(out=lnd[:], in_=side["ldet"][:], func=AF.Ln))
        hq = sing.tile([M, 1], f32)
        nc.vector.tensor_scalar(out=hq[:], in0=stats[0:M, 8:9], scalar1=0.5,
                                scalar2=float(np.log(2.0 * np.pi)),
                                op0=OP.mult, op1=OP.add)
        nc.vector.tensor_tensor(out=stats[0:M, 0:1], in0=hq[:], in1=lnd[:], op=OP.add)
        # dice den: row sums of masked C (the rz in the stationary cancels
        # the masked-exp row sums, so sum_q C[e,q] = sum_pixels true[p,e])
        nc.vector.tensor_reduce(out=stats[0:E, 3:4], in_=C_ps[0:E, 0:Q], axis=AX.X, op=OP.add)
        # dice num: sum over matched pairs of C
        scr2 = sing.tile([E, Q], f32)
        nc.vector.scalar_tensor_tensor(
            out=scr2[:], in0=C_ps[0:E, 0:Q], scalar=1.0, in1=smt[0:E, C_HS:C_HS + Q],
            op0=OP.mult, op1=OP.mult, accum_out=stats[0:E, 2:3])
        # occ x-term total: C row 96, cols 160:164 -> stats[96, 5]
        nc.vector.tensor_reduce(out=stats[E:LE, 5:6], in_=C_ps[E:LE, Q:RQ],
                                axis=AX.X, op=OP.add)

        fin_ps = ps.tile([1, 12], f32)
        nc.tensor.matmul(out=fin_ps[:], lhsT=ones[:], rhs=stats[:], start=True, stop=True)
        nc.vector.tensor_copy(out=res[:], in_=fin_ps[:])
        nc.sync.dma_start(out=partials.ap(), in_=res[:])

    nc.compile()
    return nc


def _get_nc():
    if "nc" not in _CACHE:
        _CACHE["nc"] = _build_nc()
    return _CACHE["nc"]


def make_in_maps(is_electron_logit, true_segmap, binary_mask_logits, portion_logits,
                 incidence_points, positions, chol, occupancy_logits, occupancy_true,
                 matched_q, matched_e):
    f = np.float32
    eye4 = np.eye(K, dtype=f)
    dr7 = np.arange(WIN) - WIN // 2
    in_maps = []
    for c in range(8):
        b, h = c // 2, c % 2
        sl = slice(h * HALF, (h + 1) * HALF)
        me = np.asarray(matched_e[b]).astype(np.int64)
        mq = np.asarray(matched_q[b]).astype(np.int64)

        por = np.asarray(portion_logits[b, sl], dtype=f).reshape(NPIX, Q)
        mq_mask = np.zeros(Q, dtype=f)
        mq_mask[np.asarray(matched_q[b]).astype(np.int64)] = 1.0
        por = por - 30.0 * (1.0 - mq_mask)[None, :]
        tru = np.asarray(true_segmap[b, sl], dtype=f).reshape(NPIX, E)
        occ = np.asarray(occupancy_logits[b, sl], dtype=f).reshape(NPIX, K)
        occt = np.asarray(occupancy_true[b, sl]).reshape(NPIX)
        stream = np.concatenate([por, tru, occ, eye4[occt]], axis=1)

        pts = np.asarray(incidence_points[b], dtype=f)[me]
        pix = np.floor(pts).astype(np.int64)
        rg = pix[:, 0:1] + dr7[None, :]
        cg = pix[:, 1:2] + dr7[None, :]
        valid = ((rg >= h * HALF) & (rg < (h + 1) * HALF)).astype(f)
        rl = rg - h * HALF
        flat = rl[:, :, None] * W + cg[:, None, :]
        flat = np.clip(flat, 0, NPIX - 1)
        offT = (flat * SC + Q + me[:, None, None]).astype(np.int32).reshape(M, W49)
        offB = (flat * Q + mq[:, None, None]).astype(np.int32).reshape(M, W49)
        valid49 = np.ascontiguousarray(
            np.broadcast_to(valid[:, :, None], (M, WIN, WIN))).reshape(M, W49)

        Hs = np.zeros((E, Q), dtype=f)
        Hs[me, mq] = 1.0
        ind = np.zeros(Q, dtype=f)
        ind[mq] = 1.0

        iel = np.asarray(is_electron_logit, dtype=f).reshape(B, Q)[b]
        lab = np.zeros(Q, dtype=f)
        lab[mq] = 1.0
        wgt = NO_E + (1.0 - NO_E) * lab

        def pack2(v):
            tmp = np.zeros(2 * M, dtype=f)
            tmp[:Q] = v
            return np.ascontiguousarray(tmp.reshape(2, M).T)

        iel2, lab2, w2 = pack2(iel), pack2(lab), pack2(wgt)

        chol_b = np.asarray(chol[b], dtype=f)[mq]
        cen = np.asarray(positions[b], dtype=f)[mq]

        sm = np.zeros((P, NS), dtype=f)
        sm[0:M, C_OFFT:C_OFFT + W49] = offT.view(f)
        sm[0:M, C_OFFB:C_OFFB + W49] = offB.view(f)
        sm[0:M, C_VAL:C_VAL + W49] = valid49
        sm[0:E, C_HS:C_HS + Q] = Hs
        sm[0:M, C_IEL:C_IEL + 2] = iel2
        sm[0:M, C_W:C_W + 2] = w2
        sm[0:M, C_LAB:C_LAB + 2] = lab2
        sm[0:M, C_PTS:C_PTS + 2] = pts
        sm[0:M, C_CEN:C_CEN + 2] = cen
        sm[0:M, C_CHOL + 0] = chol_b[:, 0, 0]
        sm[0:M, C_CHOL + 1] = chol_b[:, 1, 1]
        sm[0:M, C_CHOL + 2] = chol_b[:, 1, 0]
        sm[:, C_IND:C_IND + Q] = ind[None, :]

        in_maps.append(dict(
            stream=np.ascontiguousarray(stream),
            bin_sl=np.ascontiguousarray(binary_mask_logits[b, sl]).reshape(NPIX, Q),
            sm32=sm,
        ))
    return in_maps


def combine(partials_list):
    s = np.stack([np.asarray(p, dtype=np.float64).reshape(12) for p in partials_list])
    # slots: 0=nll 1=bce 2=num2 3=den_true 4=sum_lse 5=occ_xt 6=cls_sp 7=cls_xz
    class_loss = (s[0::2, 6].sum() - s[0::2, 7].sum()) / (B * Q)
    nll_loss = s[0::2, 0].sum() / (B * M)
    bce_loss = s[:, 1].sum() / (B * M * W49)
    occ_loss = (s[:, 4].sum() - s[:, 5].sum()) / (B * H * W)
    dice = 0.0
    for b in range(B):
        num = 2.0 * (s[2 * b, 2] + s[2 * b + 1, 2])
        den = s[2 * b, 3] + s[2 * b + 1, 3] + H * W
        dice += 1.0 - (num + 1.0) / (den + 1.0)
    dice_loss = dice / B
    return np.float32(class_loss + bce_loss + dice_loss + nll_loss + occ_loss)


def kernel(**inputs):
    from concourse.bass_utils import run_bass_kernel_spmd
    nc = _get_nc()
    in_maps = make_in_maps(**{k: np.asarray(v) for k, v in inputs.items()})
    r = run_bass_kernel_spmd(nc, in_maps, list(range(8)))
    return combine([r.results[c]["partials"] for c in range(8)])
